# revision 15
# baseline (speedup 1.0000x reference)
"""MoE-LoRA linear layer (T=16384, D=1024, E=64, R=8) on 8 Trainium2 cores.

Strategy: data-parallel over tokens (2048 tokens/core). Inside each core
everything is computed transposed (d on partitions, tokens on the free dim)
so every matmul consumes operands in their natural layout with no on-device
transposes:

  out_T[:, g] = sum_k W_k^T @ xT_k[:, g]      base GEMM, N=512 token groups
  out_T[:, b] += B_blk^T @ (mask_b * (A_blk^T @ xT[:, b]))   rank-8 LoRA

Routing is resolved on the host: each core's tokens are sorted by expert
label and cut into 256-token blocks; per block the (<=16) experts present
are packed into per-block A / B / mask tensors. The device program is thus
identical for all 8 cores (one SPMD NEFF) and all data-dependence lives in
input data. The LoRA matmul accumulates directly into the base GEMM's PSUM
tile (column sub-range), so composition costs no extra DVE work.

The base GEMM runs in bf16 (fp8 would halve PE time but e4m3 quantization
of x and W costs ~3.7e-2 rel err, over the 2e-2 budget). The A-side LoRA
matmul (xa = A^T x) runs in fp8 e4m3 DoubleRow mode - two k-planes per
instruction at 2x bf16 rate - with A pre-scaled by 1/4 (compensated in B)
to stay in e4m3's normal range. Group 0's fp8 x is NOT shipped: the idle
Scalar (ACT) engine casts each bf16 wave plane to fp8 on arrival, cutting
~0.5MB off the wire-limited wave phase. Output DMA is bf16 (upcast on
host), halving write traffic and the drain tail.

Schedule: the first token group's x/W[j<5] stream as k-plane wave chunks
(wave 0 split into two half-wave DMAs so the first chunk lands ~0.8us
sooner); real base matmuls start as soon as the first half-wave arrives,
with only 4 throwaway warmup matmuls bridging the framework preamble so
the HAM clock gate (1.2 -> 2.4 GHz) ramp overlaps real work instead of
padding. Each xa DoubleRow batch is emitted one half-wave later than its
data so the in-order PE queue never blocks on an fp8-table arrival. After
the waves, group 0's deferred base columns (j>=5) interleave with the
held loras in an order that always keeps >=1 column-chunk of base matmuls
queued ahead of the first consumer of each newly-arriving LoRA table
(group-0 masks ride a small early DMA; B tables follow per block).
Output DMAs ride the same Sync DGE queue as the inputs. The last group
drains per column-chunk; its final chunk is computed as two independent
256-token half-chains (separate PSUM banks) so the last output DMA issues
~1us earlier and the exposed final transfer is quarter-sized.
"""

import numpy as np
import ml_dtypes

import concourse.bacc as bacc
import concourse.mybir as mybir
from concourse import tile
from concourse.bass_utils import run_bass_kernel_spmd

T, D, E, R = 16384, 1024, 64, 8
N_CORES = 8
TPC = T // N_CORES          # tokens per core
KD = D // 128               # 8 contraction chunks
KQ = KD // 2                # k-pair waves
GRP = 512                   # base-GEMM token group (one PSUM bank)
NG = TPC // GRP             # 4 groups
SCALING = 1.0 / R
SLOTS = 128 // R            # experts per lora block the packed layout holds
A_DIV = 4.0                 # A pre-scale (exact power of 2), folded into B
JW = 5                      # W columns j<JW ride the waves; the rest follow
N_WARM = 6                  # PE warmups bridging preamble -> first wave MM

BF16 = ml_dtypes.bfloat16
F8 = ml_dtypes.float8_e4m3

_compiled = {}              # n_blocks -> Bacc program (reused across calls)
_last_in_maps = None


def _build_nc(n_blocks: int):
    blk = TPC // n_blocks   # lora block (256 default)
    sub = GRP // blk        # lora blocks per token group
    WVB = GRP + JW * 128    # bf16 wave row per k-plane: x | W[:JW]
    WVF = n_blocks * 128    # fp8 wave row per k-plane: A8 only
    WR = (KD - JW) * 128    # deferred W columns per k-plane
    LB = D + blk            # lora-table row per block (blocks >= sub): B | M
    L0 = sub * (blk + D)    # group-0 lora table row: masks[sub] | B[sub]
    bf = mybir.dt.bfloat16
    f8 = mybir.dt.float8e4
    f32 = mybir.dt.float32
    DR = mybir.MatmulPerfMode.DoubleRow

    nc = bacc.Bacc(
        "TRN2", target_bir_lowering=False, debug=False, num_devices=N_CORES
    )
    # host-packed SBUF layouts; every DMA source is contiguous per partition
    wvb_d = nc.dram_tensor("wvb", [KQ, 128, 2, WVB], bf, kind="ExternalInput")
    wvf_d = nc.dram_tensor("wvf", [KQ, 128, 2, WVF], f8, kind="ExternalInput")
    wb2_d = nc.dram_tensor("wb2", [128, KQ, 2, WR], bf, kind="ExternalInput")
    xr_d = nc.dram_tensor("xr", [NG - 1, 128, KD, GRP], bf, kind="ExternalInput")
    x8r_d = nc.dram_tensor("x8r", [NG - 1, 128, KD, GRP], f8, kind="ExternalInput")
    # group-0 lora table (masks first so they can ride an early DMA slice),
    # then the remaining groups' tables in two pieces by first use
    lt0_d = nc.dram_tensor("lt0", [128, L0], bf, kind="ExternalInput")
    lt_shapes = [sub, n_blocks - 2 * sub]
    lt_d = [
        nc.dram_tensor(f"lt{i + 1}", [128, n * LB], bf, kind="ExternalInput")
        for i, n in enumerate(lt_shapes)
    ]
    bias_d = nc.dram_tensor("bias", [128, KD], f32, kind="ExternalInput")
    out_d = nc.dram_tensor("outT", [KD, 128, TPC], bf, kind="ExternalOutput")

    with tile.TileContext(nc) as tc:
        with (
            tc.tile_pool(name="consts", bufs=1) as cpool,
            tc.tile_pool(name="xa_ps", bufs=2, space="PSUM") as xa_ps,
            tc.tile_pool(name="out_ps", bufs=6, space="PSUM") as out_ps,
            tc.tile_pool(name="stage", bufs=16) as stage_pool,
        ):
            bias_t = cpool.tile([128, KD], f32, tag="bias", name="bias_t")
            wvb_t = [
                cpool.tile([128, 2, WVB], bf, tag=f"wvb{q}", name=f"wvb_t{q}")
                for q in range(KQ)
            ]
            wvf_t = [
                cpool.tile([128, 2, WVF], f8, tag=f"wvf{q}", name=f"wvf_t{q}")
                for q in range(KQ)
            ]
            wb2_t = cpool.tile([128, KQ, 2, WR], bf, tag="wb2", name="wb2_t")
            xr_t = [
                cpool.tile([128, KD, GRP], bf, tag=f"xr{g}", name=f"xr_t{g}")
                for g in range(1, NG)
            ]
            x8r_t = [
                cpool.tile([128, KD, GRP], f8, tag=f"x8r{g}", name=f"x8r_t{g}")
                for g in range(1, NG)
            ]
            x08_t = cpool.tile([128, KD, GRP], f8, tag="x08", name="x08_t")
            lt0_t = cpool.tile([128, L0], bf, tag="lt0", name="lt0_t")
            lt_t = [
                cpool.tile([128, n * LB], bf, tag=f"lt{i + 1}", name=f"lt_t{i + 1}")
                for i, n in enumerate(lt_shapes)
            ]
            warm_sb = cpool.tile([128, GRP], bf, tag="warm", name="warm_sb")

            def a_sl(b, q):
                # DoubleRow stationary: [128 slots, 2 planes, 128]
                o = b * 128
                return wvf_t[q][:, :, o : o + 128]

            def w_sl(k, j):
                q, kk = divmod(k, 2)
                if j < JW:
                    o = GRP + j * 128
                    return wvb_t[q][:, kk, o : o + 128]
                o = (j - JW) * 128
                return wb2_t[:, q, kk, o : o + 128]

            def x_sl(g, k, c0, c1):
                if g == 0:
                    q, kk = divmod(k, 2)
                    return wvb_t[q][:, kk, c0:c1]
                return xr_t[g - 1][:, k, c0:c1]

            def x8_sl(g, q, c0, c1):
                # DoubleRow moving: [128, 2 planes, tokens]
                if g == 0:
                    return x08_t[:, 2 * q : 2 * q + 2, c0:c1]
                return x8r_t[g - 1][:, 2 * q : 2 * q + 2, c0:c1]

            def b_sl(b, j):
                if b < sub:
                    o = sub * blk + b * D + j * 128
                    return lt0_t[:, o : o + 128]
                t, lb = (lt_t[0], b - sub) if b < 2 * sub else (lt_t[1], b - 2 * sub)
                o = lb * LB + j * 128
                return t[:, o : o + 128]

            def m_sl(b):
                if b < sub:
                    return lt0_t[:, b * blk : (b + 1) * blk]
                t, lb = (lt_t[0], b - sub) if b < 2 * sub else (lt_t[1], b - 2 * sub)
                o = lb * LB + D
                return t[:, o : o + blk]

            # warm_sb memset is the first Vector op so the PE warmup chain
            # starts as early as the post-preamble barrier allows
            nc.vector.memset(warm_sb[:], 0.0)

            # issue order == arrival order per queue; inputs ride the Sync
            # queue front-loaded so late hoisted consumers never stall.
            # Wave 0 is split into per-k-plane halves and leads the queue so
            # real matmuls can start ~0.8us sooner; bias (needed late) rides
            # after it. Group-0 masks (small) ride ahead of the B tables.
            nc.sync.dma_start(wvb_t[0][:, 0, :], wvb_d[0, :, 0, :])
            nc.sync.dma_start(wvb_t[0][:, 1, :], wvb_d[0, :, 1, :])
            nc.sync.dma_start(bias_t[:], bias_d[:, :])
            nc.sync.dma_start(wvb_t[1][:], wvb_d[1, :, :, :])
            nc.sync.dma_start(wvf_t[0][:], wvf_d[0, :, :, :])
            nc.sync.dma_start(wvb_t[2][:], wvb_d[2, :, :, :])
            nc.sync.dma_start(wvf_t[1][:], wvf_d[1, :, :, :])
            nc.sync.dma_start(wvb_t[3][:], wvb_d[3, :, :, :])
            nc.sync.dma_start(wvf_t[2][:], wvf_d[2, :, :, :])
            nc.sync.dma_start(wb2_t[:], wb2_d[:, :, :, :])
            nc.sync.dma_start(lt0_t[:, : sub * blk], lt0_d[:, : sub * blk])
            nc.sync.dma_start(wvf_t[3][:], wvf_d[3, :, :, :])
            for h in range(sub):
                o = sub * blk + h * D
                nc.sync.dma_start(
                    lt0_t[:, o : o + D], lt0_d[:, o : o + D]
                )
            # group 1's x arrives in k-plane halves: its first base matmuls
            # (and the hoisted xa DRs) unblock ~0.7us before the full tile
            nc.sync.dma_start(xr_t[0][:, : KD // 2, :], xr_d[0, :, : KD // 2, :])
            nc.sync.dma_start(xr_t[0][:, KD // 2 :, :], xr_d[0, :, KD // 2 :, :])
            nc.sync.dma_start(x8r_t[0][:, : KD // 2, :], x8r_d[0, :, : KD // 2, :])
            nc.sync.dma_start(x8r_t[0][:, KD // 2 :, :], x8r_d[0, :, KD // 2 :, :])
            nc.sync.dma_start(lt_t[0][:], lt_d[0][:, :])
            nc.sync.dma_start(x8r_t[1][:], x8r_d[1, :, :, :])
            nc.sync.dma_start(x8r_t[2][:], x8r_d[2, :, :, :])
            nc.sync.dma_start(lt_t[1][:], lt_d[1][:, :])
            nc.sync.dma_start(xr_t[1][:], xr_d[1, :, :, :])
            nc.sync.dma_start(xr_t[2][:], xr_d[2, :, :, :])

            def emit_warm(n):
                for _ in range(n):
                    warm_ps = out_ps.tile([128, GRP], f32, tag="o", name="warm_ps")
                    nc.tensor.matmul(
                        warm_ps[:],
                        lhsT=warm_sb[:, 0:128],
                        rhs=warm_sb[:],
                        start=True,
                        stop=True,
                        skip_group_check=True,
                    )

            # PE warmup across the tail of the fixed framework preamble; just
            # enough to bridge until wave 0's first half lands, so the HAM
            # ramp (~3.4us of continuous activity) overlaps real work
            emit_warm(N_WARM)

            xa_m = [None] * n_blocks
            xa_p = [None] * n_blocks
            def emit_xa(b, qs, masked):
                # xa[slot, t] for lora block b over k-pair chunks qs (fp8 DR)
                # NB: each accumulator needs its own PSUM tile - sharing one
                # bank corrupts a neighbor mid-use (start=True zeroes at
                # bank granularity, not the written column range)
                g, h = divmod(b, sub)
                if qs[0] == 0:
                    xa_p[b] = xa_ps.tile([128, blk], f32, tag="xa", name=f"xa_p{b}")
                for q in qs:
                    nc.tensor.matmul(
                        xa_p[b][:],
                        lhsT=a_sl(b, q),
                        rhs=x8_sl(g, q, h * blk, (h + 1) * blk),
                        start=(q == 0),
                        stop=(q == KQ - 1),
                        perf_mode=DR,
                        skip_group_check=True,
                    )
                if masked:
                    emit_mask(b)

            def emit_mask(b):
                # separate from emit_xa so callers can defer it: a mask
                # emitted right at the xa DRs sits in the in-order Vector
                # queue waiting on them and starves the PSUM bias-drain
                # chain behind it (GpSimd can't take it - no PSUM reads)
                xm = cpool.tile([128, blk], bf, tag=f"xam{b}", name=f"xm{b}")
                nc.vector.tensor_mul(xm[:], xa_p[b][:], m_sl(b))
                xa_m[b] = xm

            def emit_base(g, j, o_p, ks):
                for k in ks:
                    nc.tensor.matmul(
                        o_p[:],
                        lhsT=w_sl(k, j),
                        rhs=x_sl(g, k, 0, GRP),
                        start=(k == 0),
                        stop=False,
                        skip_group_check=True,
                    )

            def emit_lora_bias(g, j, o_p, split=False):
                if split and sub > 1:
                    # last group: drain in block-sized pieces so the final
                    # exposed DVE+DMA is small; pieces issue from the Sync
                    # queue (same queue as inputs - descriptors dispatch to
                    # 16 parallel engines so a late out completes promptly)
                    for h in range(sub):
                        b = g * sub + h
                        nc.tensor.matmul(
                            o_p[:, h * blk : (h + 1) * blk],
                            lhsT=b_sl(b, j),
                            rhs=xa_m[b][:],
                            start=False,
                            stop=(h == sub - 1),
                            skip_group_check=True,
                        )
                    # j==KD-2 drains via the (idle by then) Scalar queue so
                    # the final half-chain's Sync-queue DMA issues aren't
                    # delayed behind these pieces
                    eng = nc.scalar if j == KD - 2 else nc.sync
                    for h in range(sub):
                        c0, c1 = h * blk, (h + 1) * blk
                        st = stage_pool.tile(
                            [128, c1 - c0], bf, tag="sts", name=f"st{g}_{j}_{h}"
                        )
                        nc.vector.tensor_scalar_add(
                            st[:], o_p[:, c0:c1], bias_t[:, j : j + 1]
                        )
                        eng.dma_start(
                            out_d[j, :, g * GRP + c0 : g * GRP + c1], st[:]
                        )
                    return
                for h in range(sub):
                    b = g * sub + h
                    nc.tensor.matmul(
                        o_p[:, h * blk : (h + 1) * blk],
                        lhsT=b_sl(b, j),
                        rhs=xa_m[b][:],
                        start=False,
                        stop=(h == sub - 1),
                        skip_group_check=True,
                    )
                st = stage_pool.tile([128, GRP], bf, tag="st", name=f"st{g}_{j}")
                nc.vector.tensor_scalar_add(
                    st[:], o_p[:], bias_t[:, j : j + 1]
                )
                nc.sync.dma_start(out_d[j, :, g * GRP : (g + 1) * GRP], st[:])

            def emit_last_j(g, j):
                # final column chunk of the final group: two independent
                # half-token chains on separate PSUM banks, so the first
                # half's bias+DMA issues while the second half's base
                # matmuls still run, and the exposed final transfer is
                # quarter-sized. The very last pieces issue from the (by
                # then idle) Scalar/GpSimd queues - safe only because the
                # DMA engines are idle at the drain.
                for h in range(sub):
                    b = g * sub + h
                    c0, c1 = h * blk, (h + 1) * blk
                    o_ph = out_ps.tile([128, GRP], f32, tag="o", name=f"o_pl{h}")
                    for k in range(KD):
                        nc.tensor.matmul(
                            o_ph[:, 0:blk],
                            lhsT=w_sl(k, j),
                            rhs=x_sl(g, k, c0, c1),
                            start=(k == 0),
                            stop=False,
                            skip_group_check=True,
                        )
                    nc.tensor.matmul(
                        o_ph[:, 0:blk],
                        lhsT=b_sl(b, j),
                        rhs=xa_m[b][:],
                        start=False,
                        stop=True,
                        skip_group_check=True,
                    )
                    if h < sub - 1:
                        st = stage_pool.tile(
                            [128, blk], bf, tag="sts", name=f"stl{h}"
                        )
                        nc.vector.tensor_scalar_add(
                            st[:], o_ph[:, 0:blk], bias_t[:, j : j + 1]
                        )
                        nc.scalar.dma_start(
                            out_d[j, :, g * GRP + c0 : g * GRP + c1], st[:]
                        )
                    else:
                        # both final pieces ride the Sync queue: a GpSimd-
                        # queue DMA takes ~2.3us issue-to-complete at the
                        # drain (vs ~1.0us on Sync) and gates teardown
                        half = blk // 2
                        for pi, (p0, p1, eng) in enumerate(
                            [(0, half, nc.sync), (half, blk, nc.sync)]
                        ):
                            st = stage_pool.tile(
                                [128, p1 - p0], bf, tag="sts", name=f"stl{h}_{pi}"
                            )
                            nc.vector.tensor_scalar_add(
                                st[:], o_ph[:, p0:p1], bias_t[:, j : j + 1]
                            )
                            eng.dma_start(
                                out_d[j, :, g * GRP + c0 + p0 : g * GRP + c0 + p1],
                                st[:],
                            )

            # --- group 0: plane-granular wave schedule matched to DMA
            # arrivals; each xa batch rides one half-wave behind its data
            # (needs JW held PSUM tiles + xa tiles; fall back to the plain
            # order for exotic block counts) ---
            if sub <= 3:
                o_p0 = {}
                for j in range(JW):
                    o_p0[j] = out_ps.tile([128, GRP], f32, tag="o", name=f"o_p0_{j}")
                for q in range(KQ):
                    for kk in range(2):
                        k = 2 * q + kk
                        for j in range(JW):
                            emit_base(0, j, o_p0[j], (k,))
                        # fp8 x for this plane, cast on the idle ACT engine
                        nc.scalar.copy(x08_t[:, k, :], x_sl(0, k, 0, GRP))
                        if kk == 0 and q > 0:
                            # previous wave's xa, one half-wave later than
                            # its last x plane so the in-order PE queue
                            # never blocks on the wvf (A8) arrival
                            for b in range(sub):
                                emit_xa(b, (q - 1,), masked=False)
                # deferred W columns (wb2 lands right after the waves) and
                # held loras, ordered so >=1 chunk of base matmuls is always
                # queued ahead of the first consumer of each newly-arriving
                # input (xa q3 / masks / B tables ride after j5's base so
                # the in-order PE queue never stalls on their arrival)
                held = list(range(JW))
                pend_p0 = None
                for j in range(JW, KD):
                    o_p = out_ps.tile([128, GRP], f32, tag="o", name=f"o_p0_{j}")
                    emit_base(0, j, o_p, range(KD))
                    if j == JW:
                        for b in range(sub):
                            emit_xa(b, (KQ - 1,), masked=True)
                    if held:
                        jh = held.pop(0)
                        emit_lora_bias(0, jh, o_p0[jh])
                    if held:
                        jh = held.pop(0)
                        emit_lora_bias(0, jh, o_p0[jh])
                    if pend_p0 is not None:
                        emit_lora_bias(0, j - 1, pend_p0)
                    if j == KD - 1:
                        # group 1's xa+masks hoisted behind j7's base: the
                        # Vector mask muls then complete a full column-chunk
                        # before group 1's first lora consumes them (emitted
                        # at group 1's j==1) instead of stalling the PE; the
                        # hoist rides after the held/pend loras so their TS
                        # ops (which recycle the PSUM ring for group 1's
                        # base tiles) aren't delayed on the in-order Vector
                        # queue behind the mask muls
                        for h in range(sub):
                            emit_xa(sub + h, range(KQ), masked=True)
                    pend_p0 = o_p
                emit_lora_bias(0, KD - 1, pend_p0)
            else:
                for kk in range(KD):
                    nc.scalar.copy(x08_t[:, kk, :], x_sl(0, kk, 0, GRP))
                for b in range(sub):
                    emit_xa(b, range(KQ), masked=True)
                for j in range(KD):
                    o_p = out_ps.tile([128, GRP], f32, tag="o", name=f"o_p0_{j}")
                    emit_base(0, j, o_p, range(KD))
                    emit_lora_bias(0, j, o_p)

            # --- groups 1..3: lora/DVE one j-step behind the base GEMM
            # (relaxes table deadlines); the next group's xa rides at j==3,
            # its masks two j-steps later so the in-order Vector queue never
            # blocks on the DRs; the last group drains immediately in
            # block-sized pieces, its final chunk as two half-chains ---
            pend = None
            for g in range(1, NG):
                for j in range(KD):
                    if g == NG - 1 and j == KD - 1:
                        if pend is not None:
                            emit_lora_bias(*pend)
                            pend = None
                        emit_last_j(g, j)
                        continue
                    o_p = out_ps.tile([128, GRP], f32, tag="o", name=f"o_p{g}_{j}")
                    emit_base(g, j, o_p, range(KD))
                    if pend is not None:
                        emit_lora_bias(*pend)
                        pend = None
                    if j == 3 and g < NG - 1:
                        for h in range(sub):
                            emit_xa((g + 1) * sub + h, range(KQ), masked=False)
                    if j == 5 and g < NG - 1:
                        for h in range(sub):
                            emit_mask((g + 1) * sub + h)
                    if g == NG - 1:
                        emit_lora_bias(g, j, o_p, split=True)
                    else:
                        pend = (g, j, o_p)

    nc.compile()
    return nc


def _pick_n_blocks(labels: np.ndarray) -> int:
    for n_blocks in (8, 16, 32, 64, 128, 256):
        blk = TPC // n_blocks
        ok = True
        for c in range(N_CORES):
            sl = np.sort(labels[c * TPC : (c + 1) * TPC])
            for b in range(n_blocks):
                if len(np.unique(sl[b * blk : (b + 1) * blk])) > SLOTS:
                    ok = False
                    break
            if not ok:
                break
        if ok:
            return n_blocks
    raise ValueError("could not find a block size with <=16 experts per block")


def kernel(x, labels, W, A, B, bias):
    global _last_in_maps
    x = np.asarray(x, dtype=np.float32)
    labels_i = np.asarray(labels).astype(np.int64)
    W = np.asarray(W, dtype=np.float32)
    A = np.asarray(A, dtype=np.float32)
    B = np.asarray(B, dtype=np.float32)
    bias = np.asarray(bias, dtype=np.float32)

    n_blocks = _pick_n_blocks(labels_i)
    blk = TPC // n_blocks
    sub = GRP // blk

    if n_blocks not in _compiled:
        _compiled[n_blocks] = _build_nc(n_blocks)
    nc = _compiled[n_blocks]

    # w_wave[q, p, kk, :] = W[128*(2q+kk)+p, :]
    w_wave = W.reshape(KQ, 2, 128, D).transpose(0, 2, 1, 3).astype(BF16)
    wb2_in = np.ascontiguousarray(
        w_wave[:, :, :, JW * 128 :].transpose(1, 0, 2, 3)  # [128, KQ, 2, WR]
    )
    bias_in = np.ascontiguousarray(bias.reshape(KD, 128).T)  # [128, KD] f32
    B_scaled = (B * (SCALING * A_DIV)).astype(np.float32)
    # A8[e]: [KQ, p, kk, r] = A[e, 128(2q+kk)+p, r] / A_DIV, fp8
    A8 = (
        (A / A_DIV).reshape(E, KQ, 2, 128, R).transpose(0, 1, 3, 2, 4).astype(F8)
    )  # [E, KQ, 128, 2, R]

    in_maps = []
    perms = []
    for c in range(N_CORES):
        lc = labels_i[c * TPC : (c + 1) * TPC]
        perm = np.argsort(lc, kind="stable")
        perms.append(perm)
        ls = lc[perm]                          # sorted labels
        xs = x[c * TPC : (c + 1) * TPC][perm]  # [TPC, D] sorted tokens

        # xt_full[k, p, g, t] = xs[g*GRP + t, 128k + p]
        xt_full = xs.astype(BF16).T.reshape(KD, 128, NG, GRP)
        xt8_full = xs.astype(F8).T.reshape(KD, 128, NG, GRP)
        # x0 wave part [q, p, kk, t]
        x0_wave = xt_full[:, :, 0, :].reshape(KQ, 2, 128, GRP).transpose(0, 2, 1, 3)
        xr_in = np.ascontiguousarray(
            xt_full[:, :, 1:, :].transpose(2, 1, 0, 3)    # [NG-1, 128, KD, GRP]
        )
        x8r_in = np.ascontiguousarray(
            xt8_full[:, :, 1:, :].transpose(2, 1, 0, 3)   # [NG-1, 128, KD, GRP]
        )

        # packed per-block expert tables
        a8_pack = np.zeros((KQ, 128, 2, n_blocks, 128), dtype=F8)
        b_pack = np.zeros((128, n_blocks, D), dtype=BF16)
        m_pack = np.zeros((128, n_blocks, blk), dtype=BF16)
        for b in range(n_blocks):
            seg = ls[b * blk : (b + 1) * blk]
            experts = np.unique(seg)
            assert len(experts) <= SLOTS
            for i, e in enumerate(experts):
                # DR lhsT slot: a8_pack[q, p, kk, b, 8i+r] = A[e,128(2q+kk)+p,r]/4
                a8_pack[:, :, :, b, i * R : (i + 1) * R] = A8[e]
                b_pack[i * R : (i + 1) * R, b, :] = B_scaled[e]
                m_pack[i * R : (i + 1) * R, b, :] = (seg == e)[None, :]

        # bf16 wave[q] = x0-pair | W[:JW]-pair ; fp8 wave[q] = A8 only
        wvb_in = np.ascontiguousarray(
            np.concatenate(
                [
                    x0_wave,                                   # [KQ,128,2,GRP]
                    w_wave[:, :, :, : JW * 128],               # [KQ,128,2,JW*128]
                ],
                axis=3,
            )
        )
        wvf_in = np.ascontiguousarray(a8_pack.reshape(KQ, 128, 2, n_blocks * 128))

        # group-0 lora table: masks first (ride an early DMA slice), then
        # per-block B tables; remaining groups as per-block [B | mask] rows
        lt0_in = np.ascontiguousarray(
            np.concatenate(
                [
                    m_pack[:, :sub].reshape(128, -1),
                    b_pack[:, :sub].reshape(128, -1),
                ],
                axis=1,
            )
        )
        lt_full = np.concatenate([b_pack, m_pack], axis=2)  # [128, nb, D+blk]
        lt_ins = [
            np.ascontiguousarray(lt_full[:, sub : 2 * sub].reshape(128, -1)),
            np.ascontiguousarray(lt_full[:, 2 * sub :].reshape(128, -1)),
        ]

        in_maps.append(
            {
                "wvb": wvb_in,
                "wvf": wvf_in,
                "wb2": wb2_in,
                "xr": xr_in,
                "x8r": x8r_in,
                "lt0": lt0_in,
                "lt1": lt_ins[0],
                "lt2": lt_ins[1],
                "bias": bias_in,
            }
        )

    _last_in_maps = in_maps
    res = run_bass_kernel_spmd(nc, in_maps, core_ids=list(range(N_CORES)))

    out = np.empty((T, D), dtype=np.float32)
    for c in range(N_CORES):
        o_t = res.results[c]["outT"].reshape(D, TPC)  # [d, t] bf16 sorted tokens
        out[c * TPC + perms[c]] = o_t.T.astype(np.float32)
    return out


# revision 22
# speedup vs baseline: 1.0118x; 1.0118x over previous
"""MoE-LoRA linear layer (T=16384, D=1024, E=64, R=8) on 8 Trainium2 cores.

Strategy: data-parallel over tokens (2048 tokens/core). Inside each core
everything is computed transposed (d on partitions, tokens on the free dim)
so every matmul consumes operands in their natural layout with no on-device
transposes:

  out_T[:, g] = sum_k W_k^T @ xT_k[:, g]      base GEMM, N=512 token groups
  out_T[:, b] += B_blk^T @ (mask_b * (A_blk^T @ xT[:, b]))   rank-8 LoRA

Routing is resolved on the host: each core's tokens are sorted by expert
label and cut into 256-token blocks; per block the (<=16) experts present
are packed into per-block A / B / mask tensors. The device program is thus
identical for all 8 cores (one SPMD NEFF) and all data-dependence lives in
input data. The LoRA matmul accumulates directly into the base GEMM's PSUM
tile (column sub-range), so composition costs no extra DVE work.

The base GEMM runs in bf16 (fp8 would halve PE time but e4m3 quantization
of x and W costs ~3.7e-2 rel err, over the 2e-2 budget). The A-side LoRA
matmul (xa = A^T x) runs in fp8 e4m3 DoubleRow mode - two k-planes per
instruction at 2x bf16 rate - with A pre-scaled by 1/4 (compensated in B)
to stay in e4m3's normal range. Group 0's fp8 x is NOT shipped: the idle
Scalar (ACT) engine casts each bf16 wave plane to fp8 on arrival, cutting
~0.5MB off the wire-limited wave phase. Output DMA is bf16 (upcast on
host), halving write traffic and the drain tail.

Schedule: the first token group's x/W[j<5] stream as k-plane wave chunks
(wave 0 split into two half-wave DMAs so the first chunk lands ~0.8us
sooner); real base matmuls start as soon as the first half-wave arrives,
with only 4 throwaway warmup matmuls bridging the framework preamble so
the HAM clock gate (1.2 -> 2.4 GHz) ramp overlaps real work instead of
padding. Each xa DoubleRow batch is emitted one half-wave later than its
data so the in-order PE queue never blocks on an fp8-table arrival. After
the waves, group 0's deferred base columns (j>=5) interleave with the
held loras in an order that always keeps >=1 column-chunk of base matmuls
queued ahead of the first consumer of each newly-arriving LoRA table
(group-0 masks ride a small early DMA; B tables follow per block).
Output DMAs ride the same Sync DGE queue as the inputs. The last group
drains per column-chunk; its final chunk is computed as two independent
256-token half-chains (separate PSUM banks) so the last output DMA issues
~1us earlier and the exposed final transfer is quarter-sized.
"""

import numpy as np
import ml_dtypes

import concourse.bacc as bacc
import concourse.mybir as mybir
from concourse import tile
from concourse.bass_utils import run_bass_kernel_spmd

T, D, E, R = 16384, 1024, 64, 8
N_CORES = 8
TPC = T // N_CORES          # tokens per core
KD = D // 128               # 8 contraction chunks
KQ = KD // 2                # k-pair waves
GRP = 512                   # base-GEMM token group (one PSUM bank)
NG = TPC // GRP             # 4 groups
SCALING = 1.0 / R
SLOTS = 128 // R            # experts per lora block the packed layout holds
A_DIV = 4.0                 # A pre-scale (exact power of 2), folded into B
JW = 5                      # W columns j<JW ride the waves; the rest follow
N_WARM = 6                  # PE warmups bridging preamble -> first wave MM

BF16 = ml_dtypes.bfloat16
F8 = ml_dtypes.float8_e4m3

_compiled = {}              # n_blocks -> Bacc program (reused across calls)
_last_in_maps = None


def _build_nc(n_blocks: int):
    blk = TPC // n_blocks   # lora block (256 default)
    sub = GRP // blk        # lora blocks per token group
    WVB = GRP + JW * 128    # bf16 wave row per k-plane: x | W[:JW]
    WVF = n_blocks * 128    # fp8 wave row per k-plane: A8 only
    WR = (KD - JW) * 128    # deferred W columns per k-plane
    LB = D + blk            # lora-table row per block (blocks >= sub): B | M
    L0 = sub * (blk + D)    # group-0 lora table row: masks[sub] | B[sub]
    bf = mybir.dt.bfloat16
    f8 = mybir.dt.float8e4
    f32 = mybir.dt.float32
    DR = mybir.MatmulPerfMode.DoubleRow

    nc = bacc.Bacc(
        "TRN2", target_bir_lowering=False, debug=False, num_devices=N_CORES
    )
    # host-packed SBUF layouts; every DMA source is contiguous per partition
    wvb_d = nc.dram_tensor("wvb", [KQ, 128, 2, WVB], bf, kind="ExternalInput")
    wvf_d = nc.dram_tensor("wvf", [KQ, 128, 2, WVF], f8, kind="ExternalInput")
    # deferred W packed j-major so each column chunk ships as its own
    # contiguous DMA, interleaved into the wave stream by first use
    wb2_d = nc.dram_tensor(
        "wb2", [KD - JW, 128, KQ, 2, 128], bf, kind="ExternalInput"
    )
    xr_d = nc.dram_tensor("xr", [NG - 1, 128, KD, GRP], bf, kind="ExternalInput")
    x8r_d = nc.dram_tensor("x8r", [NG - 1, 128, KD, GRP], f8, kind="ExternalInput")
    # group-0 lora table (masks first so they can ride an early DMA slice),
    # then the remaining groups' tables in two pieces by first use
    lt0_d = nc.dram_tensor("lt0", [128, L0], bf, kind="ExternalInput")
    lt_shapes = [sub, n_blocks - 2 * sub]
    lt_d = [
        nc.dram_tensor(f"lt{i + 1}", [128, n * LB], bf, kind="ExternalInput")
        for i, n in enumerate(lt_shapes)
    ]
    bias_d = nc.dram_tensor("bias", [128, KD], f32, kind="ExternalInput")
    out_d = nc.dram_tensor("outT", [KD, 128, TPC], bf, kind="ExternalOutput")

    with tile.TileContext(nc) as tc:
        with (
            tc.tile_pool(name="consts", bufs=1) as cpool,
            tc.tile_pool(name="xa_ps", bufs=2, space="PSUM") as xa_ps,
            tc.tile_pool(name="out_ps", bufs=6, space="PSUM") as out_ps,
            tc.tile_pool(name="stage", bufs=16) as stage_pool,
        ):
            bias_t = cpool.tile([128, KD], f32, tag="bias", name="bias_t")
            wvb_t = [
                cpool.tile([128, 2, WVB], bf, tag=f"wvb{q}", name=f"wvb_t{q}")
                for q in range(KQ)
            ]
            wvf_t = [
                cpool.tile([128, 2, WVF], f8, tag=f"wvf{q}", name=f"wvf_t{q}")
                for q in range(KQ)
            ]
            wb2_t = [
                cpool.tile([128, KQ, 2, 128], bf, tag=f"wb2_{jj}", name=f"wb2_t{jj}")
                for jj in range(KD - JW)
            ]
            xr_t = [
                cpool.tile([128, KD, GRP], bf, tag=f"xr{g}", name=f"xr_t{g}")
                for g in range(1, NG)
            ]
            x8r_t = [
                cpool.tile([128, KD, GRP], f8, tag=f"x8r{g}", name=f"x8r_t{g}")
                for g in range(1, NG)
            ]
            x08_t = cpool.tile([128, KD, GRP], f8, tag="x08", name="x08_t")
            lt0_t = cpool.tile([128, L0], bf, tag="lt0", name="lt0_t")
            lt_t = [
                cpool.tile([128, n * LB], bf, tag=f"lt{i + 1}", name=f"lt_t{i + 1}")
                for i, n in enumerate(lt_shapes)
            ]
            warm_sb = cpool.tile([128, GRP], bf, tag="warm", name="warm_sb")

            def a_sl(b, q):
                # DoubleRow stationary: [128 slots, 2 planes, 128]
                o = b * 128
                return wvf_t[q][:, :, o : o + 128]

            def w_sl(k, j):
                q, kk = divmod(k, 2)
                if j < JW:
                    o = GRP + j * 128
                    return wvb_t[q][:, kk, o : o + 128]
                return wb2_t[j - JW][:, q, kk, :]

            def x_sl(g, k, c0, c1):
                if g == 0:
                    q, kk = divmod(k, 2)
                    return wvb_t[q][:, kk, c0:c1]
                return xr_t[g - 1][:, k, c0:c1]

            def x8_sl(g, q, c0, c1):
                # DoubleRow moving: [128, 2 planes, tokens]
                if g == 0:
                    return x08_t[:, 2 * q : 2 * q + 2, c0:c1]
                return x8r_t[g - 1][:, 2 * q : 2 * q + 2, c0:c1]

            def b_sl(b, j):
                if b < sub:
                    o = sub * blk + b * D + j * 128
                    return lt0_t[:, o : o + 128]
                t, lb = (lt_t[0], b - sub) if b < 2 * sub else (lt_t[1], b - 2 * sub)
                o = lb * LB + j * 128
                return t[:, o : o + 128]

            def m_sl(b):
                if b < sub:
                    return lt0_t[:, b * blk : (b + 1) * blk]
                t, lb = (lt_t[0], b - sub) if b < 2 * sub else (lt_t[1], b - 2 * sub)
                o = lb * LB + D
                return t[:, o : o + blk]

            # warm_sb memset is the first Vector op so the PE warmup chain
            # starts as early as the post-preamble barrier allows
            nc.vector.memset(warm_sb[:], 0.0)

            # issue order == arrival order per queue; inputs ride the Sync
            # queue front-loaded so late hoisted consumers never stall.
            # Wave 0 is split into per-k-plane halves and leads the queue so
            # real matmuls can start ~0.8us sooner; bias (needed late) rides
            # after it. Group-0 masks (small) ride ahead of the B tables.
            nc.sync.dma_start(wvb_t[0][:, 0, :], wvb_d[0, :, 0, :])
            nc.sync.dma_start(wvb_t[0][:, 1, :], wvb_d[0, :, 1, :])
            nc.sync.dma_start(bias_t[:], bias_d[:, :])
            nc.sync.dma_start(wvb_t[1][:], wvb_d[1, :, :, :])
            nc.sync.dma_start(wvf_t[0][:], wvf_d[0, :, :, :])
            nc.sync.dma_start(wvb_t[2][:], wvb_d[2, :, :, :])
            nc.sync.dma_start(wvf_t[1][:], wvf_d[1, :, :, :])
            nc.sync.dma_start(wvb_t[3][:], wvb_d[3, :, :, :])
            # deferred W chunks interleave with the remaining tables in
            # consumption order, so j5's base matmuls never wait ~2us for
            # one big wb2 transfer behind the whole wave stream
            nc.sync.dma_start(wb2_t[0][:], wb2_d[0, :, :, :, :])
            nc.sync.dma_start(wvf_t[2][:], wvf_d[2, :, :, :])
            nc.sync.dma_start(lt0_t[:, : sub * blk], lt0_d[:, : sub * blk])
            nc.sync.dma_start(wb2_t[1][:], wb2_d[1, :, :, :, :])
            nc.sync.dma_start(wvf_t[3][:], wvf_d[3, :, :, :])
            for h in range(sub):
                o = sub * blk + h * D
                nc.sync.dma_start(
                    lt0_t[:, o : o + D], lt0_d[:, o : o + D]
                )
            nc.sync.dma_start(wb2_t[2][:], wb2_d[2, :, :, :, :])
            # group 1's x arrives in k-plane halves: its first base matmuls
            # and the staged xa DRs unblock ~0.7us before the full tiles
            nc.sync.dma_start(xr_t[0][:, : KD // 2, :], xr_d[0, :, : KD // 2, :])
            nc.sync.dma_start(x8r_t[0][:, : KD // 2, :], x8r_d[0, :, : KD // 2, :])
            nc.sync.dma_start(xr_t[0][:, KD // 2 :, :], xr_d[0, :, KD // 2 :, :])
            nc.sync.dma_start(x8r_t[0][:, KD // 2 :, :], x8r_d[0, :, KD // 2 :, :])
            nc.sync.dma_start(lt_t[0][:], lt_d[0][:, :])
            nc.sync.dma_start(x8r_t[1][:], x8r_d[1, :, :, :])
            nc.sync.dma_start(x8r_t[2][:], x8r_d[2, :, :, :])
            nc.sync.dma_start(lt_t[1][:], lt_d[1][:, :])
            nc.sync.dma_start(xr_t[1][:], xr_d[1, :, :, :])
            nc.sync.dma_start(xr_t[2][:], xr_d[2, :, :, :])

            def emit_warm(n):
                for _ in range(n):
                    warm_ps = out_ps.tile([128, GRP], f32, tag="o", name="warm_ps")
                    nc.tensor.matmul(
                        warm_ps[:],
                        lhsT=warm_sb[:, 0:128],
                        rhs=warm_sb[:],
                        start=True,
                        stop=True,
                        skip_group_check=True,
                    )

            # PE warmup across the tail of the fixed framework preamble; just
            # enough to bridge until wave 0's first half lands, so the HAM
            # ramp (~3.4us of continuous activity) overlaps real work
            emit_warm(N_WARM)

            xa_m = [None] * n_blocks
            xa_p = [None] * n_blocks
            def emit_xa(b, qs, masked):
                # xa[slot, t] for lora block b over k-pair chunks qs (fp8 DR)
                # NB: each accumulator needs its own PSUM tile - sharing one
                # bank corrupts a neighbor mid-use (start=True zeroes at
                # bank granularity, not the written column range)
                g, h = divmod(b, sub)
                if qs[0] == 0:
                    xa_p[b] = xa_ps.tile([128, blk], f32, tag="xa", name=f"xa_p{b}")
                for q in qs:
                    nc.tensor.matmul(
                        xa_p[b][:],
                        lhsT=a_sl(b, q),
                        rhs=x8_sl(g, q, h * blk, (h + 1) * blk),
                        start=(q == 0),
                        stop=(q == KQ - 1),
                        perf_mode=DR,
                        skip_group_check=True,
                    )
                if masked:
                    emit_mask(b)

            def emit_mask(b):
                # separate from emit_xa so callers can defer it: a mask
                # emitted right at the xa DRs sits in the in-order Vector
                # queue waiting on them and starves the PSUM bias-drain
                # chain behind it (GpSimd can't take it - no PSUM reads)
                xm = cpool.tile([128, blk], bf, tag=f"xam{b}", name=f"xm{b}")
                nc.vector.tensor_mul(xm[:], xa_p[b][:], m_sl(b))
                xa_m[b] = xm

            def emit_base(g, j, o_p, ks):
                for k in ks:
                    nc.tensor.matmul(
                        o_p[:],
                        lhsT=w_sl(k, j),
                        rhs=x_sl(g, k, 0, GRP),
                        start=(k == 0),
                        stop=False,
                        skip_group_check=True,
                    )

            def emit_lora_bias(g, j, o_p, split=False):
                if split and sub > 1:
                    # last group: drain in block-sized pieces so the final
                    # exposed DVE+DMA is small; pieces issue from the Sync
                    # queue (same queue as inputs - descriptors dispatch to
                    # 16 parallel engines so a late out completes promptly)
                    for h in range(sub):
                        b = g * sub + h
                        nc.tensor.matmul(
                            o_p[:, h * blk : (h + 1) * blk],
                            lhsT=b_sl(b, j),
                            rhs=xa_m[b][:],
                            start=False,
                            stop=(h == sub - 1),
                            skip_group_check=True,
                        )
                    # j==KD-2 drains via the (idle by then) Scalar queue so
                    # the final half-chain's Sync-queue DMA issues aren't
                    # delayed behind these pieces
                    eng = nc.scalar if j == KD - 2 else nc.sync
                    for h in range(sub):
                        c0, c1 = h * blk, (h + 1) * blk
                        st = stage_pool.tile(
                            [128, c1 - c0], bf, tag="sts", name=f"st{g}_{j}_{h}"
                        )
                        nc.vector.tensor_scalar_add(
                            st[:], o_p[:, c0:c1], bias_t[:, j : j + 1]
                        )
                        eng.dma_start(
                            out_d[j, :, g * GRP + c0 : g * GRP + c1], st[:]
                        )
                    return
                for h in range(sub):
                    b = g * sub + h
                    nc.tensor.matmul(
                        o_p[:, h * blk : (h + 1) * blk],
                        lhsT=b_sl(b, j),
                        rhs=xa_m[b][:],
                        start=False,
                        stop=(h == sub - 1),
                        skip_group_check=True,
                    )
                st = stage_pool.tile([128, GRP], bf, tag="st", name=f"st{g}_{j}")
                nc.vector.tensor_scalar_add(
                    st[:], o_p[:], bias_t[:, j : j + 1]
                )
                nc.sync.dma_start(out_d[j, :, g * GRP : (g + 1) * GRP], st[:])

            def emit_last_j(g, j):
                # final column chunk of the final group: two independent
                # half-token chains on separate PSUM banks, so the first
                # half's bias+DMA issues while the second half's base
                # matmuls still run, and the exposed final transfer is
                # quarter-sized. The very last pieces issue from the (by
                # then idle) Scalar/GpSimd queues - safe only because the
                # DMA engines are idle at the drain.
                for h in range(sub):
                    b = g * sub + h
                    c0, c1 = h * blk, (h + 1) * blk
                    o_ph = out_ps.tile([128, GRP], f32, tag="o", name=f"o_pl{h}")
                    for k in range(KD):
                        nc.tensor.matmul(
                            o_ph[:, 0:blk],
                            lhsT=w_sl(k, j),
                            rhs=x_sl(g, k, c0, c1),
                            start=(k == 0),
                            stop=False,
                            skip_group_check=True,
                        )
                    nc.tensor.matmul(
                        o_ph[:, 0:blk],
                        lhsT=b_sl(b, j),
                        rhs=xa_m[b][:],
                        start=False,
                        stop=True,
                        skip_group_check=True,
                    )
                    if h < sub - 1:
                        st = stage_pool.tile(
                            [128, blk], bf, tag="sts", name=f"stl{h}"
                        )
                        nc.vector.tensor_scalar_add(
                            st[:], o_ph[:, 0:blk], bias_t[:, j : j + 1]
                        )
                        nc.scalar.dma_start(
                            out_d[j, :, g * GRP + c0 : g * GRP + c1], st[:]
                        )
                    else:
                        # both final pieces ride the Sync queue: a GpSimd-
                        # queue DMA takes ~2.3us issue-to-complete at the
                        # drain (vs ~1.0us on Sync) and gates teardown
                        half = blk // 2
                        for pi, (p0, p1, eng) in enumerate(
                            [(0, half, nc.sync), (half, blk, nc.sync)]
                        ):
                            st = stage_pool.tile(
                                [128, p1 - p0], bf, tag="sts", name=f"stl{h}_{pi}"
                            )
                            nc.vector.tensor_scalar_add(
                                st[:], o_ph[:, p0:p1], bias_t[:, j : j + 1]
                            )
                            eng.dma_start(
                                out_d[j, :, g * GRP + c0 + p0 : g * GRP + c0 + p1],
                                st[:],
                            )

            # --- group 0: plane-granular wave schedule matched to DMA
            # arrivals; each xa batch rides one half-wave behind its data
            # (needs JW held PSUM tiles + xa tiles; fall back to the plain
            # order for exotic block counts) ---
            if sub <= 3:
                o_p0 = {}
                for j in range(JW):
                    o_p0[j] = out_ps.tile([128, GRP], f32, tag="o", name=f"o_p0_{j}")
                for q in range(KQ):
                    for kk in range(2):
                        k = 2 * q + kk
                        for j in range(JW):
                            emit_base(0, j, o_p0[j], (k,))
                        # fp8 x for this plane, cast on the idle ACT engine
                        nc.scalar.copy(x08_t[:, k, :], x_sl(0, k, 0, GRP))
                        if kk == 0 and q > 0:
                            # previous wave's xa, one half-wave later than
                            # its last x plane so the in-order PE queue
                            # never blocks on the wvf (A8) arrival
                            for b in range(sub):
                                emit_xa(b, (q - 1,), masked=False)
                # deferred W columns (wb2 lands right after the waves) and
                # held loras, ordered so >=1 chunk of base matmuls is always
                # queued ahead of the first consumer of each newly-arriving
                # input (xa q3 / masks / B tables ride after j5's base so
                # the in-order PE queue never stalls on their arrival)
                held = list(range(JW))
                pend_p0 = None
                for j in range(JW, KD):
                    o_p = out_ps.tile([128, GRP], f32, tag="o", name=f"o_p0_{j}")
                    emit_base(0, j, o_p, range(KD))
                    if j == JW:
                        for b in range(sub):
                            emit_xa(b, (KQ - 1,), masked=True)
                    if held:
                        jh = held.pop(0)
                        emit_lora_bias(0, jh, o_p0[jh])
                    if held:
                        jh = held.pop(0)
                        emit_lora_bias(0, jh, o_p0[jh])
                    if pend_p0 is not None:
                        emit_lora_bias(0, j - 1, pend_p0)
                    if j == KD - 1:
                        # group 1's xa hoisted behind j7's base, staged to
                        # the k-half arrivals of its fp8 x: q0-1 here, q2-3
                        # (+ masks) after group 1's first base chunk. The
                        # hoist rides after the held/pend loras so their TS
                        # ops (which recycle the PSUM ring for group 1's
                        # base tiles) aren't delayed on the in-order Vector
                        # queue behind the mask muls
                        for h in range(sub):
                            emit_xa(sub + h, (0, 1), masked=False)
                    pend_p0 = o_p
                emit_lora_bias(0, KD - 1, pend_p0)
            else:
                for kk in range(KD):
                    nc.scalar.copy(x08_t[:, kk, :], x_sl(0, kk, 0, GRP))
                for b in range(sub):
                    emit_xa(b, range(KQ), masked=True)
                for j in range(KD):
                    o_p = out_ps.tile([128, GRP], f32, tag="o", name=f"o_p0_{j}")
                    emit_base(0, j, o_p, range(KD))
                    emit_lora_bias(0, j, o_p)

            # --- groups 1..3: lora/DVE one j-step behind the base GEMM
            # (relaxes table deadlines); the next group's xa rides at j==3,
            # its masks two j-steps later so the in-order Vector queue never
            # blocks on the DRs; the last group drains immediately in
            # block-sized pieces, its final chunk as two half-chains ---
            pend = None
            for g in range(1, NG):
                for j in range(KD):
                    if g == NG - 1 and j == KD - 1:
                        if pend is not None:
                            emit_lora_bias(*pend)
                            pend = None
                        emit_last_j(g, j)
                        continue
                    o_p = out_ps.tile([128, GRP], f32, tag="o", name=f"o_p{g}_{j}")
                    emit_base(g, j, o_p, range(KD))
                    if g == 1 and j == 0:
                        # second half of group 1's staged xa (see above)
                        for h in range(sub):
                            emit_xa(sub + h, (2, 3), masked=True)
                    if pend is not None:
                        emit_lora_bias(*pend)
                        pend = None
                    if j == 3 and g < NG - 1:
                        for h in range(sub):
                            emit_xa((g + 1) * sub + h, range(KQ), masked=False)
                    if j == 5 and g < NG - 1:
                        for h in range(sub):
                            emit_mask((g + 1) * sub + h)
                    if g == NG - 1:
                        emit_lora_bias(g, j, o_p, split=True)
                    else:
                        pend = (g, j, o_p)

    nc.compile()
    return nc


def _pick_n_blocks(labels: np.ndarray) -> int:
    for n_blocks in (8, 16, 32, 64, 128, 256):
        blk = TPC // n_blocks
        ok = True
        for c in range(N_CORES):
            sl = np.sort(labels[c * TPC : (c + 1) * TPC])
            for b in range(n_blocks):
                if len(np.unique(sl[b * blk : (b + 1) * blk])) > SLOTS:
                    ok = False
                    break
            if not ok:
                break
        if ok:
            return n_blocks
    raise ValueError("could not find a block size with <=16 experts per block")


def kernel(x, labels, W, A, B, bias):
    global _last_in_maps
    x = np.asarray(x, dtype=np.float32)
    labels_i = np.asarray(labels).astype(np.int64)
    W = np.asarray(W, dtype=np.float32)
    A = np.asarray(A, dtype=np.float32)
    B = np.asarray(B, dtype=np.float32)
    bias = np.asarray(bias, dtype=np.float32)

    n_blocks = _pick_n_blocks(labels_i)
    blk = TPC // n_blocks
    sub = GRP // blk

    if n_blocks not in _compiled:
        _compiled[n_blocks] = _build_nc(n_blocks)
    nc = _compiled[n_blocks]

    # w_wave[q, p, kk, :] = W[128*(2q+kk)+p, :]
    w_wave = W.reshape(KQ, 2, 128, D).transpose(0, 2, 1, 3).astype(BF16)
    # deferred W j-major: wb2_in[jj, p, q, kk, :] = W chunk j=JW+jj
    wb2_in = np.ascontiguousarray(
        w_wave[:, :, :, JW * 128 :]                      # [KQ, 128, 2, WR]
        .reshape(KQ, 128, 2, KD - JW, 128)
        .transpose(3, 1, 0, 2, 4)                        # [KD-JW, 128, KQ, 2, 128]
    )
    bias_in = np.ascontiguousarray(bias.reshape(KD, 128).T)  # [128, KD] f32
    B_scaled = (B * (SCALING * A_DIV)).astype(np.float32)
    # A8[e]: [KQ, p, kk, r] = A[e, 128(2q+kk)+p, r] / A_DIV, fp8
    A8 = (
        (A / A_DIV).reshape(E, KQ, 2, 128, R).transpose(0, 1, 3, 2, 4).astype(F8)
    )  # [E, KQ, 128, 2, R]

    in_maps = []
    perms = []
    for c in range(N_CORES):
        lc = labels_i[c * TPC : (c + 1) * TPC]
        perm = np.argsort(lc, kind="stable")
        perms.append(perm)
        ls = lc[perm]                          # sorted labels
        xs = x[c * TPC : (c + 1) * TPC][perm]  # [TPC, D] sorted tokens

        # xt_full[k, p, g, t] = xs[g*GRP + t, 128k + p]
        xt_full = xs.astype(BF16).T.reshape(KD, 128, NG, GRP)
        xt8_full = xs.astype(F8).T.reshape(KD, 128, NG, GRP)
        # x0 wave part [q, p, kk, t]
        x0_wave = xt_full[:, :, 0, :].reshape(KQ, 2, 128, GRP).transpose(0, 2, 1, 3)
        xr_in = np.ascontiguousarray(
            xt_full[:, :, 1:, :].transpose(2, 1, 0, 3)    # [NG-1, 128, KD, GRP]
        )
        x8r_in = np.ascontiguousarray(
            xt8_full[:, :, 1:, :].transpose(2, 1, 0, 3)   # [NG-1, 128, KD, GRP]
        )

        # packed per-block expert tables
        a8_pack = np.zeros((KQ, 128, 2, n_blocks, 128), dtype=F8)
        b_pack = np.zeros((128, n_blocks, D), dtype=BF16)
        m_pack = np.zeros((128, n_blocks, blk), dtype=BF16)
        for b in range(n_blocks):
            seg = ls[b * blk : (b + 1) * blk]
            experts = np.unique(seg)
            assert len(experts) <= SLOTS
            for i, e in enumerate(experts):
                # DR lhsT slot: a8_pack[q, p, kk, b, 8i+r] = A[e,128(2q+kk)+p,r]/4
                a8_pack[:, :, :, b, i * R : (i + 1) * R] = A8[e]
                b_pack[i * R : (i + 1) * R, b, :] = B_scaled[e]
                m_pack[i * R : (i + 1) * R, b, :] = (seg == e)[None, :]

        # bf16 wave[q] = x0-pair | W[:JW]-pair ; fp8 wave[q] = A8 only
        wvb_in = np.ascontiguousarray(
            np.concatenate(
                [
                    x0_wave,                                   # [KQ,128,2,GRP]
                    w_wave[:, :, :, : JW * 128],               # [KQ,128,2,JW*128]
                ],
                axis=3,
            )
        )
        wvf_in = np.ascontiguousarray(a8_pack.reshape(KQ, 128, 2, n_blocks * 128))

        # group-0 lora table: masks first (ride an early DMA slice), then
        # per-block B tables; remaining groups as per-block [B | mask] rows
        lt0_in = np.ascontiguousarray(
            np.concatenate(
                [
                    m_pack[:, :sub].reshape(128, -1),
                    b_pack[:, :sub].reshape(128, -1),
                ],
                axis=1,
            )
        )
        lt_full = np.concatenate([b_pack, m_pack], axis=2)  # [128, nb, D+blk]
        lt_ins = [
            np.ascontiguousarray(lt_full[:, sub : 2 * sub].reshape(128, -1)),
            np.ascontiguousarray(lt_full[:, 2 * sub :].reshape(128, -1)),
        ]

        in_maps.append(
            {
                "wvb": wvb_in,
                "wvf": wvf_in,
                "wb2": wb2_in,
                "xr": xr_in,
                "x8r": x8r_in,
                "lt0": lt0_in,
                "lt1": lt_ins[0],
                "lt2": lt_ins[1],
                "bias": bias_in,
            }
        )

    _last_in_maps = in_maps
    res = run_bass_kernel_spmd(nc, in_maps, core_ids=list(range(N_CORES)))

    out = np.empty((T, D), dtype=np.float32)
    for c in range(N_CORES):
        o_t = res.results[c]["outT"].reshape(D, TPC)  # [d, t] bf16 sorted tokens
        out[c * TPC + perms[c]] = o_t.T.astype(np.float32)
    return out
